# revision 13
# baseline (speedup 1.0000x reference)
"""ContrastiveTokenLoss on Trainium2 (8 NeuronCores, Bass/Tile).

Problem (hardcoded): input [2, 2048, 32000] f32 logits, target [2, 2048] int.
ct_len = round(2048*0.25) = 512, win = round(512*0.5) = 256,
IGNORE_INDEX = -100, PAD_ID = 0.

loss = sum_{b, i<512} valid(b,i) * log1p( sum_{j in [i-256, i), tgt[b,j]!=0}
           exp(x[b,i,tgt[b,j]] - x[b,i,tgt_safe[b,i]]) ) / max(#valid, 1)

Sharding: the 512 contrastive positions are split across the 8 cores (64 per
core per batch element; positions >= 512 are never touched).  Each core
receives its slab of logits laid out vocab-major ([32001, 128]: one vocab row
per window token is then contiguous; row 32000 is a -1e9 sentinel used to
mask PAD negatives), a packed int16 index table (own targets + the [64k-256,
64k+64) target window for both batch elements, in dma_gather's 16-partition
wrapped layout), its own 128 targets, and one constant table (additive band
mask | identity).

On-device per core: one dma_gather pulls the 768 rows the loss touches
(512 B contiguous each, ~384 KB instead of the 16 MB slab), PE transposes the
six [128,128] chunks back to (position-partition, window-free) layout, DVE
adds the -1e9 band mask straight out of PSUM, ACT computes a fused
exp+row-sum with the positive logit as a per-partition bias, then log1p and
one [128,2]x[128,1] matmul reduce loss / valid-count over partitions.  Each
core returns [loss_sum, valid_count]; the host sums 8 partials and divides.
"""

import numpy as np
from contextlib import ExitStack

import concourse.bass as bass
import concourse.bacc as bacc
import concourse.mybir as mybir
import concourse.tile as tile
from concourse.bass_utils import run_bass_kernel_spmd

B, T, V = 2, 2048, 32000
CT = 512
WIN = 256
IGNORE_INDEX = -100
PAD_ID = 0
NCORES = 8
CI = CT // NCORES          # 64 positions per core per batch
W = WIN + CI               # 320 window positions per core per batch
NW = B * W                 # 640 window rows per core
NG = P = B * CI            # 128 partition rows = (batch, local position)
NIDX = P + NW              # 768 gathered rows (own-target chunk + window)
NCH = NIDX // P            # 6 chunks
IDXC = NIDX // 16          # 48 idx-table columns (16-partition wrap)
F32 = mybir.dt.float32
I16 = mybir.dt.int16
I32 = mybir.dt.int32

_CACHE = {}


def _build():
    nc = bacc.Bacc("TRN2", target_bir_lowering=False)
    xt = nc.dram_tensor("xt", [V + 1, P], F32, kind="ExternalInput")
    idx = nc.dram_tensor("idx", [P, IDXC], I16, kind="ExternalInput")
    to = nc.dram_tensor("to", [P, 1], I32, kind="ExternalInput")
    cst = nc.dram_tensor("cst", [P, NW + P], F32, kind="ExternalInput")
    out = nc.dram_tensor("out", [2, 1], F32, kind="ExternalOutput")

    with ExitStack() as ctx:
        tc = ctx.enter_context(tile.TileContext(nc))
        sb = ctx.enter_context(tc.tile_pool(name="sb", bufs=1))
        ps = ctx.enter_context(tc.tile_pool(name="ps", bufs=1, space="PSUM"))

        # index table first so the gather can start as early as possible
        it = sb.tile([P, IDXC], I16)
        nc.sync.dma_start(it[:], idx[:])
        # clamp (tgt_safe) ...
        nc.vector.tensor_scalar(it[:], it[:], 0, None, mybir.AluOpType.max)
        # ... and redirect PAD window tokens (cols 8: are window rows) to the
        # -1e9 sentinel row V so their exp contribution vanishes
        adj = sb.tile([P, IDXC - P // 16], I16)
        nc.vector.tensor_scalar(
            adj[:], it[:, P // 16 :], 0, V, mybir.AluOpType.is_equal,
            mybir.AluOpType.mult,
        )
        nc.vector.tensor_tensor(
            it[:, P // 16 :], it[:, P // 16 :], adj[:], mybir.AluOpType.add
        )

        # one gather for all 768 rows: g3[p, c, :] = xt[idx_flat[c*128+p], :]
        g3 = sb.tile([P, NCH, P], F32)
        nc.gpsimd.dma_gather(
            out_ap=g3[:],
            in_ap=xt[:],
            idxs_ap=it[:],
            num_idxs=NIDX,
            num_idxs_reg=NIDX,
            elem_size=P,
        )

        cst_sb = sb.tile([P, NW + P], F32)
        nc.sync.dma_start(cst_sb[:], cst[:])
        mb = cst_sb[:, 0:NW]
        ident = cst_sb[:, NW : NW + P]
        to_sb = sb.tile([P, 1], I32)
        nc.sync.dma_start(to_sb[:], to[:])

        # transpose each gathered chunk; consume straight out of PSUM
        gm = sb.tile([P, NW], F32)
        pd = sb.tile([P, P], F32)
        for c in range(NCH):
            pt = ps.tile([P, P], F32, tag=f"pt{c}", space="PSUM")
            nc.tensor.transpose(out=pt[:], in_=g3[:, c, :], identity=ident)
            if c == 0:
                # chunk 0 rows are the own targets: pos[p] = pt[p, p]
                nc.vector.tensor_tensor(pd[:], pt[:], ident, mybir.AluOpType.mult)
            else:
                sl = slice((c - 1) * P, c * P)
                nc.vector.tensor_tensor(
                    gm[:, sl], pt[:], mb[:, sl], mybir.AluOpType.add
                )
        npos = sb.tile([P, 1], F32)
        nc.vector.reduce_sum(npos[:], pd[:], axis=mybir.AxisListType.X, negate=True)

        # fused exp(gm - pos) with per-row accumulation
        e = sb.tile([P, NW], F32)
        r = sb.tile([P, 1], F32)
        nc.scalar.activation(
            e[:], gm[:], mybir.ActivationFunctionType.Exp,
            bias=npos[:], scale=1.0, accum_out=r[:],
        )

        # lv[:, 0] = ln(1 + r) * valid ; lv[:, 1] = valid
        lv = sb.tile([P, 2], F32)
        nc.scalar.activation(
            lv[:, 0:1], r[:], mybir.ActivationFunctionType.Ln, bias=1.0, scale=1.0
        )
        nc.vector.tensor_scalar(
            lv[:, 1:2], to_sb[:], IGNORE_INDEX, None, mybir.AluOpType.not_equal
        )
        nc.vector.tensor_mul(lv[:, 0:1], lv[:, 0:1], lv[:, 1:2])

        # partition reduction: out[2, 1] = lv.T @ ones
        ones = sb.tile([P, 1], F32)
        nc.vector.memset(ones[:], 1.0)
        acc = ps.tile([2, 1], F32, space="PSUM")
        nc.tensor.matmul(out=acc[:], lhsT=lv[:], rhs=ones[:], start=True, stop=True)
        res = sb.tile([2, 1], F32)
        nc.vector.tensor_copy(res[:], acc[:])
        nc.sync.dma_start(out[:], res[:])
    nc.compile()
    return nc


def _get_nc():
    if "nc" not in _CACHE:
        _CACHE["nc"] = _build()
    return _CACHE["nc"]


def _consts():
    if "consts" not in _CACHE:
        p = np.arange(P, dtype=np.int64)
        il = (p % CI)[:, None]
        bp = (p // CI)[:, None]
        f = np.arange(NW, dtype=np.int64)[None, :]
        jl = f % W
        bf = f // W
        band = (bf == bp) & (jl >= il) & (jl < il + WIN)
        cstv = np.full((P, NW + P), -1e9, np.float32)
        cstv[:, 0:NW][band] = 0.0
        cstv[:, NW:] = np.eye(P, dtype=np.float32)
        sent = np.full((1, P), -1e9, np.float32)
        _CACHE["consts"] = (np.ascontiguousarray(cstv), sent)
    return _CACHE["consts"]


def kernel(input, target, _trace=False):
    input = np.asarray(input, dtype=np.float32)
    target = np.asarray(target)
    cstv, sent = _consts()
    t32 = target[:, :CT].astype(np.int32)

    n = np.arange(NIDX)
    in_maps = []
    for k in range(NCORES):
        s = k * CI
        lo = s - WIN
        if lo >= 0:
            twk = t32[:, lo : s + CI]
        else:
            twk = np.concatenate(
                [np.zeros((B, -lo), np.int32), t32[:, : s + CI]], axis=1
            )
        tok = t32[:, s : s + CI].reshape(-1)
        ids_flat = np.concatenate([tok, twk.reshape(-1)]).astype(np.int16)
        # dma_gather's wrapped idx layout, replicated for the 8 gpsimd cores
        blk = np.zeros((16, IDXC), np.int16)
        blk[n % 16, n // 16] = ids_flat
        idxs = np.tile(blk, (8, 1))
        xtk = np.empty((V + 1, P), np.float32)
        xtk[:V] = input[:, s : s + CI, :].reshape(P, V).T
        xtk[V:] = sent
        in_maps.append(
            {
                "xt": xtk,
                "idx": idxs,
                "to": np.ascontiguousarray(tok).reshape(P, 1),
                "cst": cstv,
            }
        )

    nc = _get_nc()
    br = run_bass_kernel_spmd(
        nc, in_maps, core_ids=list(range(NCORES)), trace=_trace
    )
    rs = np.stack([r["out"] for r in br.results])  # [8, 2, 1]
    loss_sum = rs[:, 0, 0].astype(np.float64).sum()
    cnt = rs[:, 1, 0].astype(np.float64).sum()
    kernel.last_results = br
    return np.asarray(np.float32(loss_sum / max(cnt, 1.0)))


# revision 18
# speedup vs baseline: 1.4525x; 1.4525x over previous
"""ContrastiveTokenLoss on Trainium2 (8 NeuronCores, Bass/Tile).

Problem (hardcoded): input [2, 2048, 32000] f32 logits, target [2, 2048] int.
ct_len = round(2048*0.25) = 512, win = round(512*0.5) = 256,
IGNORE_INDEX = -100, PAD_ID = 0.

loss = sum_{b, i<512} valid(b,i) * log1p( sum_{j in [i-256, i), tgt[b,j]!=0}
           exp(x[b,i,tgt[b,j]] - x[b,i,tgt_safe[b,i]]) ) / max(#valid, 1)

Sharding: the 512 contrastive positions are split across the 8 cores (64 per
core per batch element; positions >= 512 are never touched).  Each core
receives its slab of logits laid out vocab-major ([32001, 128]: one vocab row
per window token is then contiguous; row 32000 is a -1e9 sentinel used to
mask PAD negatives), a packed int16 index table (own targets + the [64k-256,
64k+64) target window for both batch elements, in dma_gather's 16-partition
wrapped layout), its own 128 targets, and one constant table (additive band
mask | identity).

On-device per core: one dma_gather pulls the 768 rows the loss touches
(512 B contiguous each, ~384 KB instead of the 16 MB slab), PE transposes the
six [128,128] chunks back to (position-partition, window-free) layout, DVE
adds the -1e9 band mask straight out of PSUM, ACT computes a fused
exp+row-sum with the positive logit as a per-partition bias, then log1p and
one [128,2]x[128,1] matmul reduce loss / valid-count over partitions.  Each
core returns [loss_sum, valid_count]; the host sums 8 partials and divides.
"""

import numpy as np
from contextlib import ExitStack

import concourse.bass as bass
import concourse.bacc as bacc
import concourse.mybir as mybir
import concourse.tile as tile
from concourse.bass_utils import run_bass_kernel_spmd

B, T, V = 2, 2048, 32000
CT = 512
WIN = 256
IGNORE_INDEX = -100
PAD_ID = 0
NCORES = 8
CI = CT // NCORES          # 64 positions per core per batch
W = WIN + CI               # 320 window positions per core per batch
NW = B * W                 # 640 window rows per core
NG = P = B * CI            # 128 partition rows = (batch, local position)
NIDX = P + NW              # 768 gathered rows (own-target chunk + window)
NCH = NIDX // P            # 6 chunks
F32 = mybir.dt.float32
I32 = mybir.dt.int32

_CACHE = {}


def _build():
    nc = bacc.Bacc("TRN2", target_bir_lowering=False)
    xt = nc.dram_tensor("xt", [V + 1, P], F32, kind="ExternalInput")
    idx = nc.dram_tensor("idx", [P, NCH], I32, kind="ExternalInput")
    cst = nc.dram_tensor("cst", [P, NW + P], F32, kind="ExternalInput")
    out = nc.dram_tensor("out", [2, 1], F32, kind="ExternalOutput")

    with ExitStack() as ctx:
        tc = ctx.enter_context(tile.TileContext(nc))
        sb = ctx.enter_context(tc.tile_pool(name="sb", bufs=1))
        ps = ctx.enter_context(tc.tile_pool(name="ps", bufs=1, space="PSUM"))

        # index table first so the gathers can start as early as possible.
        # column c holds the row indices of gather chunk c: chunk 0 = own
        # targets (for pos), chunks 1..5 = the window tokens.
        it = sb.tile([P, NCH], I32)
        nc.sync.dma_start(it[:], idx[:])
        # valid = (own target != IGNORE_INDEX), read before clamping
        lv = sb.tile([P, 2], F32)
        nc.vector.tensor_scalar(
            lv[:, 1:2], it[:, 0:1], IGNORE_INDEX, None, mybir.AluOpType.not_equal
        )
        # clamp (tgt_safe) ...
        nc.vector.tensor_scalar(it[:], it[:], 0, None, mybir.AluOpType.max)
        # ... and redirect PAD window tokens (cols 1..5) to the -1e9 sentinel
        # row V so their exp contribution vanishes
        adj = sb.tile([P, NCH - 1], I32)
        nc.vector.tensor_scalar(
            adj[:], it[:, 1:], 0, V, mybir.AluOpType.is_equal, mybir.AluOpType.mult
        )
        nc.vector.tensor_tensor(it[:, 1:], it[:, 1:], adj[:], mybir.AluOpType.add)

        cst_sb = sb.tile([P, NW + P], F32)
        nc.sync.dma_start(cst_sb[:], cst[:])
        mb = cst_sb[:, 0:NW]
        ident = cst_sb[:, NW : NW + P]

        # gather chunk by chunk; transpose and consume straight out of PSUM
        gm = sb.tile([P, NW], F32)
        pd = sb.tile([P, P], F32)
        for c in range(NCH):
            gt = sb.tile([P, P], F32, tag=f"gt{c}")
            nc.gpsimd.indirect_dma_start(
                out=gt[:],
                out_offset=None,
                in_=xt[:],
                in_offset=bass.IndirectOffsetOnAxis(ap=it[:, c : c + 1], axis=0),
            )
            pt = ps.tile([P, P], F32, tag=f"pt{c}", space="PSUM")
            nc.tensor.transpose(out=pt[:], in_=gt[:], identity=ident)
            if c == 0:
                # chunk 0 rows are the own targets: pos[p] = pt[p, p]
                nc.vector.tensor_tensor(pd[:], pt[:], ident, mybir.AluOpType.mult)
            else:
                sl = slice((c - 1) * P, c * P)
                nc.vector.tensor_tensor(
                    gm[:, sl], pt[:], mb[:, sl], mybir.AluOpType.add
                )
        npos = sb.tile([P, 1], F32)
        nc.vector.reduce_sum(npos[:], pd[:], axis=mybir.AxisListType.X, negate=True)

        # fused exp(gm - pos) with per-row accumulation
        e = sb.tile([P, NW], F32)
        r = sb.tile([P, 1], F32)
        nc.scalar.activation(
            e[:], gm[:], mybir.ActivationFunctionType.Exp,
            bias=npos[:], scale=1.0, accum_out=r[:],
        )

        # lv[:, 0] = ln(1 + r) * valid ; lv[:, 1] = valid (set earlier)
        nc.scalar.activation(
            lv[:, 0:1], r[:], mybir.ActivationFunctionType.Ln, bias=1.0, scale=1.0
        )
        nc.vector.tensor_mul(lv[:, 0:1], lv[:, 0:1], lv[:, 1:2])

        # partition reduction: out[2, 1] = lv.T @ ones
        ones = sb.tile([P, 1], F32)
        nc.vector.memset(ones[:], 1.0)
        acc = ps.tile([2, 1], F32, space="PSUM")
        nc.tensor.matmul(out=acc[:], lhsT=lv[:], rhs=ones[:], start=True, stop=True)
        res = sb.tile([2, 1], F32)
        nc.vector.tensor_copy(res[:], acc[:])
        nc.sync.dma_start(out[:], res[:])
    nc.compile()
    return nc


def _get_nc():
    if "nc" not in _CACHE:
        _CACHE["nc"] = _build()
    return _CACHE["nc"]


def _consts():
    if "consts" not in _CACHE:
        p = np.arange(P, dtype=np.int64)
        il = (p % CI)[:, None]
        bp = (p // CI)[:, None]
        f = np.arange(NW, dtype=np.int64)[None, :]
        jl = f % W
        bf = f // W
        band = (bf == bp) & (jl >= il) & (jl < il + WIN)
        cstv = np.full((P, NW + P), -1e9, np.float32)
        cstv[:, 0:NW][band] = 0.0
        cstv[:, NW:] = np.eye(P, dtype=np.float32)
        sent = np.full((1, P), -1e9, np.float32)
        _CACHE["consts"] = (np.ascontiguousarray(cstv), sent)
    return _CACHE["consts"]


def kernel(input, target, _trace=False):
    input = np.asarray(input, dtype=np.float32)
    target = np.asarray(target)
    cstv, sent = _consts()
    t32 = target[:, :CT].astype(np.int32)

    in_maps = []
    for k in range(NCORES):
        s = k * CI
        lo = s - WIN
        if lo >= 0:
            twk = t32[:, lo : s + CI]
        else:
            twk = np.concatenate(
                [np.zeros((B, -lo), np.int32), t32[:, : s + CI]], axis=1
            )
        tok = t32[:, s : s + CI].reshape(-1)
        ids_flat = np.concatenate([tok, twk.reshape(-1)]).astype(np.int32)
        idxs = np.ascontiguousarray(ids_flat.reshape(NCH, P).T)  # [128, 6]
        xtk = np.empty((V + 1, P), np.float32)
        xtk[:V] = input[:, s : s + CI, :].reshape(P, V).T
        xtk[V:] = sent
        in_maps.append({"xt": xtk, "idx": idxs, "cst": cstv})

    nc = _get_nc()
    br = run_bass_kernel_spmd(
        nc, in_maps, core_ids=list(range(NCORES)), trace=_trace
    )
    rs = np.stack([r["out"] for r in br.results])  # [8, 2, 1]
    loss_sum = rs[:, 0, 0].astype(np.float64).sum()
    cnt = rs[:, 1, 0].astype(np.float64).sum()
    kernel.last_results = br
    return np.asarray(np.float32(loss_sum / max(cnt, 1.0)))


# revision 19
# speedup vs baseline: 1.5031x; 1.0348x over previous
"""ContrastiveTokenLoss on Trainium2 (8 NeuronCores, Bass/Tile).

Problem (hardcoded): input [2, 2048, 32000] f32 logits, target [2, 2048] int.
ct_len = round(2048*0.25) = 512, win = round(512*0.5) = 256,
IGNORE_INDEX = -100, PAD_ID = 0.

loss = sum_{b, i<512} valid(b,i) * log1p( sum_{j in [i-256, i), tgt[b,j]!=0}
           exp(x[b,i,tgt[b,j]] - x[b,i,tgt_safe[b,i]]) ) / max(#valid, 1)

Sharding: the 512 contrastive positions are split across the 8 cores (64 per
core per batch element; positions >= 512 are never touched).  Each core
receives its slab of logits laid out vocab-major ([32001, 128]: the one vocab
row the loss needs per window token is then contiguous; row 32000 is a -1e9
sentinel that PAD window tokens are redirected to, which zeroes their exp
contribution), the per-chunk gather row indices, its own 128 targets, and one
constant table (additive -1e9 band mask | 128x128 identity).

On-device per core: 5 (6 when PAD/ignore tokens are present) indirect DMAs
gather the window rows the loss touches (512 B contiguous each, ~320 KB
instead of the 16 MB slab), PE transposes each [128,128] chunk back to
(position-partition, window-free) layout, DVE adds the band mask straight
out of PSUM and extracts the positive logit from the window diagonal, ACT
computes a fused exp+row-sum with -pos as a per-partition bias, then log1p
and one [128,2]x[128,1] matmul reduce loss / valid-count over partitions.
Each core returns [loss_sum, valid_count]; the host sums the 8 partials and
divides.
"""

import numpy as np
from contextlib import ExitStack

import concourse.bass as bass
import concourse.bacc as bacc
import concourse.mybir as mybir
import concourse.tile as tile
from concourse.bass_utils import run_bass_kernel_spmd

B, T, V = 2, 2048, 32000
CT = 512
WIN = 256
IGNORE_INDEX = -100
PAD_ID = 0
NCORES = 8
CI = CT // NCORES          # 64 positions per core per batch
W = WIN + CI               # 320 window positions per core per batch
NW = B * W                 # 640 window rows per core
P = B * CI                 # 128 partition rows = (batch, local position)
F32 = mybir.dt.float32
I32 = mybir.dt.int32

_CACHE = {}


def _build(pos_chunk):
    """pos_chunk=False: 5 gathers, pos read off the window diagonal (exact
    when every target in [0, CT) is > 0, which the host checks).
    pos_chunk=True: an extra leading gather chunk holds the own-target rows
    unsentineled so PAD own-targets still produce the right pos."""
    nch = 6 if pos_chunk else 5
    nc = bacc.Bacc("TRN2", target_bir_lowering=False)
    xt = nc.dram_tensor("xt", [V + 1, P], F32, kind="ExternalInput")
    idx = nc.dram_tensor("idx", [P, nch], I32, kind="ExternalInput")
    to = nc.dram_tensor("to", [P, 1], I32, kind="ExternalInput")
    cst = nc.dram_tensor("cst", [P, NW + P], F32, kind="ExternalInput")
    out = nc.dram_tensor("out", [2, 1], F32, kind="ExternalOutput")

    with ExitStack() as ctx:
        tc = ctx.enter_context(tile.TileContext(nc))
        sb = ctx.enter_context(tc.tile_pool(name="sb", bufs=1))
        ps = ctx.enter_context(tc.tile_pool(name="ps", bufs=1, space="PSUM"))

        # gather row indices first (host pre-clamped and pre-sentineled) so
        # the gathers start as early as possible
        it = sb.tile([P, nch], I32)
        nc.sync.dma_start(it[:], idx[:])

        cst_sb = sb.tile([P, NW + P], F32)
        nc.sync.dma_start(cst_sb[:], cst[:])
        mb = cst_sb[:, 0:NW]
        ident = cst_sb[:, NW : NW + P]
        to_sb = sb.tile([P, 1], I32)
        nc.sync.dma_start(to_sb[:], to[:])

        # gather chunk by chunk; transpose and consume straight out of PSUM
        gm = sb.tile([P, NW], F32)
        pd = sb.tile([P, P], F32)
        npos = sb.tile([P, 1], F32)
        pts = []
        for c in range(nch):
            gt = sb.tile([P, P], F32, tag=f"gt{c}")
            nc.gpsimd.indirect_dma_start(
                out=gt[:],
                out_offset=None,
                in_=xt[:],
                in_offset=bass.IndirectOffsetOnAxis(ap=it[:, c : c + 1], axis=0),
            )
            pt = ps.tile([P, P], F32, tag=f"pt{c}", space="PSUM")
            nc.tensor.transpose(out=pt[:], in_=gt[:], identity=ident)
            pts.append(pt)
            w = c - 1 if pos_chunk else c  # window chunk number
            if pos_chunk and c == 0:
                # chunk 0 rows are the own targets: pos[p] = pt[p, p]
                nc.vector.tensor_tensor(pd[:], pt[:], ident, mybir.AluOpType.mult)
            else:
                sl = slice(w * P, (w + 1) * P)
                nc.vector.tensor_tensor(
                    gm[:, sl], pt[:], mb[:, sl], mybir.AluOpType.add
                )
        if not pos_chunk:
            # pos sits on the window diagonal: window chunk 2 rows 0..63
            # (batch 0) and window chunk 4 rows 64..127 (batch 1)
            h = P // B
            nc.vector.tensor_tensor(
                pd[0:h, :], pts[2][0:h, :], ident[0:h, :], mybir.AluOpType.mult
            )
            nc.vector.tensor_tensor(
                pd[h:P, :], pts[4][h:P, :], ident[h:P, :], mybir.AluOpType.mult
            )
        nc.vector.reduce_sum(npos[:], pd[:], axis=mybir.AxisListType.X, negate=True)

        # fused exp(gm - pos) with per-row accumulation
        e = sb.tile([P, NW], F32)
        r = sb.tile([P, 1], F32)
        nc.scalar.activation(
            e[:], gm[:], mybir.ActivationFunctionType.Exp,
            bias=npos[:], scale=1.0, accum_out=r[:],
        )

        # lv[:, 0] = ln(1 + r) * valid ; lv[:, 1] = valid
        lv = sb.tile([P, 2], F32)
        nc.scalar.activation(
            lv[:, 0:1], r[:], mybir.ActivationFunctionType.Ln, bias=1.0, scale=1.0
        )
        nc.vector.tensor_scalar(
            lv[:, 1:2], to_sb[:], IGNORE_INDEX, None, mybir.AluOpType.not_equal
        )
        nc.vector.tensor_mul(lv[:, 0:1], lv[:, 0:1], lv[:, 1:2])

        # partition reduction: out[2, 1] = lv.T @ ones
        ones = sb.tile([P, 1], F32)
        nc.vector.memset(ones[:], 1.0)
        acc = ps.tile([2, 1], F32, space="PSUM")
        nc.tensor.matmul(out=acc[:], lhsT=lv[:], rhs=ones[:], start=True, stop=True)
        res = sb.tile([2, 1], F32)
        nc.vector.tensor_copy(res[:], acc[:])
        nc.sync.dma_start(out[:], res[:])
    nc.compile()
    return nc


def _get_nc(pos_chunk):
    key = f"nc{pos_chunk}"
    if key not in _CACHE:
        _CACHE[key] = _build(pos_chunk)
    return _CACHE[key]


def _consts():
    if "consts" not in _CACHE:
        p = np.arange(P, dtype=np.int64)
        il = (p % CI)[:, None]
        bp = (p // CI)[:, None]
        f = np.arange(NW, dtype=np.int64)[None, :]
        jl = f % W
        bf = f // W
        band = (bf == bp) & (jl >= il) & (jl < il + WIN)
        cstv = np.full((P, NW + P), -1e9, np.float32)
        cstv[:, 0:NW][band] = 0.0
        cstv[:, NW:] = np.eye(P, dtype=np.float32)
        # diag positions in the flat window: j == i rows per batch
        diagf = np.zeros(NW, bool)
        diagf[WIN : W] = True
        diagf[W + WIN : 2 * W] = True
        _CACHE["consts"] = (np.ascontiguousarray(cstv), diagf)
    return _CACHE["consts"]


def kernel(input, target, _trace=False):
    input = np.asarray(input, dtype=np.float32)
    target = np.asarray(target)
    cstv, diagf = _consts()
    t32 = target[:, :CT].astype(np.int32)

    # fast path: pos can be read off the window diagonal iff no target in the
    # contrastive range is PAD (0) or negative
    pos_chunk = bool((t32 <= 0).any())

    in_maps = []
    for k in range(NCORES):
        s = k * CI
        lo = s - WIN
        if lo >= 0:
            twk = t32[:, lo : s + CI]
        else:
            twk = np.concatenate(
                [np.zeros((B, -lo), np.int32), t32[:, : s + CI]], axis=1
            )
        tok = t32[:, s : s + CI].reshape(-1)
        win_ids = twk.reshape(-1)
        # tgt_safe clamp + redirect PAD window tokens to the sentinel row V
        # (the own-target/diagonal copies stay clamped so pos is exact)
        safe = np.maximum(win_ids, 0)
        sent_ids = np.where(win_ids == PAD_ID, V, safe)
        if pos_chunk:
            ids_flat = np.concatenate([np.maximum(tok, 0), sent_ids])
        else:
            ids_flat = np.where(diagf, safe, sent_ids)
        nch = ids_flat.size // P
        idxs = np.ascontiguousarray(ids_flat.reshape(nch, P).T)
        xtk = np.empty((V + 1, P), np.float32)
        xtk[:V] = input[:, s : s + CI, :].reshape(P, V).T
        xtk[V:] = -1e9
        in_maps.append(
            {
                "xt": xtk,
                "idx": idxs,
                "to": np.ascontiguousarray(tok).reshape(P, 1),
                "cst": cstv,
            }
        )

    nc = _get_nc(pos_chunk)
    br = run_bass_kernel_spmd(
        nc, in_maps, core_ids=list(range(NCORES)), trace=_trace
    )
    rs = np.stack([r["out"] for r in br.results])  # [8, 2, 1]
    loss_sum = rs[:, 0, 0].astype(np.float64).sum()
    cnt = rs[:, 1, 0].astype(np.float64).sum()
    kernel.last_results = br
    return np.asarray(np.float32(loss_sum / max(cnt, 1.0)))
